# revision 12
# baseline (speedup 1.0000x reference)
"""Trainium2 Bass kernel for GroupedQueryAttention (inverted sliding-window mask + sink).

Full inputs in, full output out. Internally head-sharded across 8 NeuronCores:
core c handles q heads {2c, 2c+1} and kv head c//2, computes its partial
(x @ Wqkv_slice -> RoPE -> scores -> masked softmax w/ sink -> AV -> @ Wo_slice),
host sums the 8 partial outputs (the all-reduce).
"""

import os
import sys
from contextlib import ExitStack

sys.path.insert(0, "/opt/trn_rl_repo")

# jax must see the axon/neuron platform; a stray JAX_PLATFORMS=cpu would hide it.
if os.environ.get("JAX_PLATFORMS", "") == "cpu":
    os.environ["JAX_PLATFORMS"] = ""

import numpy as np

import concourse.bass as bass
import concourse.tile as tile
from concourse import bacc, mybir

F32 = mybir.dt.float32
F32R = mybir.dt.float32r

N_CORES = 8
L = 2048
D = 2048
HD = 128
WINDOW = 1024
ROPE_BASE = 1024.0
SM_SCALE = 1.0 / float(np.sqrt(HD))
MASK_VAL = -1.0e5

QB = 512          # q block (free dim of score tiles)
NQB = L // QB     # 4
NKT = L // HD     # 16 k tiles of 128
NDK = D // HD     # 16 contraction chunks for projections
NLB = L // QB     # 4 l-blocks for projection

# additive-mask tiles are keyed by diff0 = q0 - k0 of the (k-tile, q-block) pair
MASK_DIFF0S = [0, -128, -256, -384, 640, 768, 896, 1024]
MASK_IDX = {d: i for i, d in enumerate(MASK_DIFF0S)}


def _classify(kt: int, qb: int):
    """masked band is 0 <= q-k <= WINDOW-1 (those entries get -inf)."""
    d0 = QB * qb - HD * kt
    if 128 <= d0 <= 512:
        return "skip", None      # tile entirely inside the band -> contributes 0
    if d0 <= -512 or d0 >= 1152:
        return "full", None      # tile entirely outside the band -> no mask needed
    return "partial", MASK_IDX[d0]


def _build_program():
    nc = bacc.Bacc("TRN2", target_bir_lowering=False, debug=False,
                   num_devices=N_CORES)

    xT_d = nc.dram_tensor("xT", [D, L], F32R, kind="ExternalInput").ap()
    wslc_d = nc.dram_tensor("wslc", [D, 4 * HD], F32R, kind="ExternalInput").ap()
    wo_d = nc.dram_tensor("wo", [2 * HD, D], F32R, kind="ExternalInput").ap()
    snk_d = nc.dram_tensor("snk", [1, 2], F32, kind="ExternalInput").ap()
    cosd_d = nc.dram_tensor("cosd", [128, L], F32, kind="ExternalInput").ap()
    sind_d = nc.dram_tensor("sind", [128, L], F32, kind="ExternalInput").ap()
    y_d = nc.dram_tensor("y", [L, D], F32, kind="ExternalOutput").ap()

    with tile.TileContext(nc) as tc, ExitStack() as stk:
        persist = stk.enter_context(tc.tile_pool(name="persist", bufs=1))

        # ---- persistent SBUF tensors ----
        wslc_sb = persist.tile([128, NDK, 4 * HD], F32R, tag="wslc")
        wo_sb = persist.tile([128, 2, D], F32R, tag="wo")
        qT = [persist.tile([128, L], F32R, tag=f"qT{h}", name=f"qT{h}") for h in range(2)]
        kT = persist.tile([128, L], F32R, tag="kT")
        vT = persist.tile([128, L], F32, tag="vT")
        v_sb = persist.tile([128, NKT, HD], F32R, tag="v")
        oT = [persist.tile([128, L], F32R, tag=f"oT{h}", name=f"oT{h}") for h in range(2)]
        cosd_sb = persist.tile([128, L], F32, tag="cosd")
        sind_sb = persist.tile([128, L], F32, tag="sind")
        masks = persist.tile([128, len(MASK_DIFF0S), QB], F32, tag="masks")
        ident = persist.tile([128, 128], F32, tag="ident")
        ones_f32 = persist.tile([128, 1], F32, tag="onesf")
        ones_sb = persist.tile([128, 1], F32R, tag="ones")
        snk_sb = persist.tile([1, 2], F32, tag="snk")
        exps_sb = persist.tile([1, 2], F32, tag="exps")

        # ---- constant / setup ops ----
        for k in range(NDK):
            nc.gpsimd.dma_start(wslc_sb[:, k, :], wslc_d[k * 128:(k + 1) * 128, :])
        nc.gpsimd.dma_start(cosd_sb[:], cosd_d[:])
        nc.gpsimd.dma_start(sind_sb[:], sind_d[:])
        nc.gpsimd.dma_start(snk_sb[:], snk_d[:])
        for h in range(2):
            nc.gpsimd.dma_start(wo_sb[:, h, :], wo_d[h * 128:(h + 1) * 128, :])

        nc.gpsimd.memset(ones_f32[:], 1.0)
        nc.scalar.copy(ones_sb[:], ones_f32[:])
        # identity for PE transposes
        nc.gpsimd.memset(ident[:], 0.0)
        nc.gpsimd.affine_select(
            out=ident[:], in_=ident[:], compare_op=mybir.AluOpType.not_equal,
            fill=1.0, base=0, channel_multiplier=1, pattern=[[-1, 128]])
        # additive mask tiles: -1e5 where 0 <= (q-k) <= WINDOW-1, else 0
        for i, d0 in enumerate(MASK_DIFF0S):
            m = masks[:, i, :]
            nc.gpsimd.memset(m, 0.0)
            # keep 0 where k-q-1 >= 0 (i.e. q-k < 0), else fill MASK_VAL
            nc.gpsimd.affine_select(
                out=m, in_=m, compare_op=mybir.AluOpType.is_ge,
                fill=MASK_VAL, base=-d0 - 1, channel_multiplier=1,
                pattern=[[-1, QB]])
            # keep where (WINDOW-1)-(q-k) >= 0 (i.e. q-k < WINDOW), else fill 0
            nc.gpsimd.affine_select(
                out=m, in_=m, compare_op=mybir.AluOpType.is_ge,
                fill=0.0, base=WINDOW - 1 - d0, channel_multiplier=1,
                pattern=[[-1, QB]])
        # exp of the two sink logits
        nc.scalar.activation(exps_sb[:], snk_sb[:], mybir.ActivationFunctionType.Exp)

        # ================= Phase A: QKV projection (transposed) =================
        # pT[c*128+r, l] = sum_d wslc[d, c*128+r] * x[l, d];  cols c = q0,q1,k,v
        col_dst = [qT[0], qT[1], kT, vT]
        with tc.tile_pool(name="psA", bufs=8, space="PSUM") as psA, \
             tc.tile_pool(name="xt", bufs=4) as xt_pool:
            for lb in range(NLB):
                psums = [psA.tile([128, QB], F32, tag="proj", name=f"psproj{c}") for c in range(4)]
                for k in range(NDK):
                    xt = xt_pool.tile([128, QB], F32R, tag="xt")
                    nc.sync.dma_start(
                        xt[:], xT_d[k * 128:(k + 1) * 128, lb * QB:(lb + 1) * QB])
                    for c in range(4):
                        nc.tensor.matmul(
                            psums[c][:],
                            wslc_sb[:, k, c * 128:(c + 1) * 128],
                            xt[:],
                            start=(k == 0), stop=(k == NDK - 1))
                for c in range(4):
                    nc.scalar.copy(col_dst[c][:, lb * QB:(lb + 1) * QB], psums[c][:])

            # ---- RoPE on qT[0], qT[1], kT (in place, transposed layout) ----
            with tc.tile_pool(name="rope", bufs=2) as rope_pool:
                for t in (qT[0], qT[1], kT):
                    partner = rope_pool.tile([128, L], F32, tag="partner")
                    nc.gpsimd.memset(partner[32:64, :], 0.0)
                    nc.gpsimd.memset(partner[96:128, :], 0.0)
                    nc.gpsimd.dma_start(partner[0:32, :], t[64:96, :].bitcast(F32))
                    nc.gpsimd.dma_start(partner[64:96, :], t[0:32, :].bitcast(F32))
                    tmp = rope_pool.tile([128, L], F32, tag="ropetmp")
                    nc.vector.tensor_mul(tmp[:], t[:], cosd_sb[:])
                    nc.vector.tensor_mul(partner[:], partner[:], sind_sb[:])
                    nc.vector.tensor_add(t[:], tmp[:], partner[:])

            # ---- v: transpose to natural (k, d) tiles ----
            for t in range(NKT):
                pt = psA.tile([128, 128], F32, tag="proj")
                nc.tensor.transpose(pt[:], vT[:, t * 128:(t + 1) * 128], ident[:])
                nc.scalar.copy(v_sb[:, t, :], pt[:])

        # ============ Phase B+C: attention + output projection ============
        with tc.tile_pool(name="psS", bufs=2, space="PSUM") as psS, \
             tc.tile_pool(name="psO", bufs=2, space="PSUM") as psO, \
             tc.tile_pool(name="psD", bufs=2, space="PSUM") as psD, \
             tc.tile_pool(name="psY", bufs=2, space="PSUM") as psY, \
             tc.tile_pool(name="sbB", bufs=6) as sbB, \
             tc.tile_pool(name="sbY", bufs=4) as sbY:
            for qb in range(NQB):
                qs = slice(qb * QB, (qb + 1) * QB)
                for h in range(2):
                    acts = [(kt, _classify(kt, qb)) for kt in range(NKT)]
                    acts = [(kt, c, mi) for kt, (c, mi) in acts if c != "skip"]
                    n_act = len(acts)
                    psum_o = psO.tile([128, QB], F32, tag="o")
                    psum_den = psD.tile([1, QB], F32, tag="den")
                    for i, (kt, cls, mi) in enumerate(acts):
                        psum_s = psS.tile([128, QB], F32, tag="s")
                        nc.tensor.matmul(
                            psum_s[:],
                            kT[:, kt * 128:(kt + 1) * 128],
                            qT[h][:, qs],
                            start=True, stop=True)
                        e_sb = sbB.tile([128, QB], F32R, tag="e")
                        if cls == "partial":
                            s_sb = sbB.tile([128, QB], F32, tag="smask")
                            nc.vector.tensor_add(s_sb[:], psum_s[:], masks[:, mi, :])
                            src = s_sb
                        else:
                            src = psum_s
                        nc.scalar.activation(
                            e_sb[:], src[:], mybir.ActivationFunctionType.Exp,
                            scale=SM_SCALE)
                        nc.tensor.matmul(
                            psum_den[:], ones_sb[:],
                            e_sb[:],
                            start=(i == 0), stop=(i == n_act - 1))
                        nc.tensor.matmul(
                            psum_o[:], v_sb[:, kt, :],
                            e_sb[:],
                            start=(i == 0), stop=(i == n_act - 1))
                    den_sb = sbB.tile([1, QB], F32, tag="densb")
                    nc.scalar.activation(
                        den_sb[:], psum_den[:],
                        mybir.ActivationFunctionType.Identity,
                        bias=exps_sb[0:1, h:h + 1])
                    r_sb = sbB.tile([1, QB], F32, tag="rsb")
                    nc.vector.reciprocal(r_sb[:], den_sb[:])
                    rb = sbB.tile([128, QB], F32, tag="rb")
                    nc.gpsimd.partition_broadcast(rb[:], r_sb[:])
                    nc.vector.tensor_mul(oT[h][:, qs], psum_o[:], rb[:])

                # ---- Wo for this q block ----
                for j in range(QB // 128):
                    qt = qb * (QB // 128) + j
                    qts = slice(qt * 128, (qt + 1) * 128)
                    for nb in range(D // QB):
                        ns = slice(nb * QB, (nb + 1) * QB)
                        psum_y = psY.tile([128, QB], F32, tag="y")
                        for h in range(2):
                            nc.tensor.matmul(
                                psum_y[:],
                                oT[h][:, qts],
                                wo_sb[:, h, ns],
                                start=(h == 0), stop=(h == 1))
                        y_sb = sbY.tile([128, QB], F32, tag="ysb")
                        if (qt + nb) % 2 == 0:
                            nc.scalar.copy(y_sb[:], psum_y[:])
                        else:
                            nc.vector.tensor_copy(y_sb[:], psum_y[:])
                        nc.sync.dma_start(y_d[qts, ns], y_sb[:])

    nc.compile()
    return nc


def _rope_tables():
    freqs = (1.0 / ROPE_BASE) ** np.linspace(0.0, 1.0, num=HD // 4,
                                             dtype=np.float32)
    theta = freqs[:, None].astype(np.float32) * np.arange(L, dtype=np.float32)[None, :]
    cos32 = np.cos(theta).astype(np.float32)
    sin32 = np.sin(theta).astype(np.float32)
    cosd = np.ones((128, L), dtype=np.float32)
    sind = np.zeros((128, L), dtype=np.float32)
    cosd[0:32] = cos32
    cosd[64:96] = cos32
    sind[0:32] = sin32
    sind[64:96] = -sin32
    return cosd, sind


def _make_in_maps(x, Wqkv, Wo, s):
    x = np.asarray(x, dtype=np.float32)
    Wqkv = np.asarray(Wqkv, dtype=np.float32)
    Wo = np.asarray(Wo, dtype=np.float32)
    s = np.asarray(s, dtype=np.float32)
    xT = np.ascontiguousarray(x.reshape(L, D).T)
    cosd, sind = _rope_tables()
    in_maps = []
    for c in range(N_CORES):
        g = c // 2
        wslc = np.concatenate([
            Wqkv[:, (2 * c) * HD:(2 * c + 2) * HD],
            Wqkv[:, 16 * HD + g * HD:16 * HD + (g + 1) * HD],
            Wqkv[:, 20 * HD + g * HD:20 * HD + (g + 1) * HD],
        ], axis=1)
        in_maps.append({
            "xT": xT,
            "wslc": np.ascontiguousarray(wslc),
            "wo": np.ascontiguousarray(Wo[(2 * c) * HD:(2 * c + 2) * HD, :]),
            "snk": np.ascontiguousarray(s[:, 2 * c:2 * c + 2]),
            "cosd": cosd,
            "sind": sind,
        })
    return in_maps


_CACHE = {}


def _get_exec():
    """Build the program once and return a cached jitted 8-core executor."""
    if "exec" in _CACHE:
        return _CACHE["exec"]

    import jax
    from jax.sharding import Mesh, PartitionSpec
    from jax.experimental.shard_map import shard_map
    from concourse.bass2jax import (_bass_exec_p, install_neuronx_cc_hook,
                                    partition_id_tensor)

    nc = _build_program()
    install_neuronx_cc_hook()

    partition_name = (nc.partition_id_tensor.name
                      if nc.partition_id_tensor else None)
    in_names, out_names, out_avals = [], [], []
    for alloc in nc.m.functions[0].allocations:
        if not isinstance(alloc, mybir.MemoryLocationSet):
            continue
        name = alloc.memorylocations[0].name
        if alloc.kind == "ExternalInput":
            if name != partition_name:
                in_names.append(name)
        elif alloc.kind == "ExternalOutput":
            out_names.append(name)
            out_avals.append(jax.core.ShapedArray(
                tuple(alloc.tensor_shape), mybir.dt.np(alloc.dtype)))
    n_params = len(in_names)
    all_names = in_names + out_names
    if partition_name is not None:
        all_names = all_names + [partition_name]

    def _body(*args):
        operands = list(args)
        if partition_name is not None:
            operands.append(partition_id_tensor())
        outs = _bass_exec_p.bind(
            *operands,
            out_avals=tuple(out_avals),
            in_names=tuple(all_names),
            out_names=tuple(out_names),
            lowering_input_output_aliases=(),
            sim_require_finite=True,
            sim_require_nnan=True,
            nc=nc,
        )
        return tuple(outs)

    devices = jax.devices()[:N_CORES]
    mesh = Mesh(np.asarray(devices), ("core",))
    n_outs = len(out_names)
    sharded = jax.jit(
        shard_map(_body, mesh=mesh,
                  in_specs=(PartitionSpec("core"),) * (n_params + n_outs),
                  out_specs=(PartitionSpec("core"),) * n_outs,
                  check_rep=False),
        keep_unused=True)

    state = {
        "sharded": sharded, "in_names": in_names, "out_names": out_names,
        "out_avals": out_avals, "mesh": mesh, "n_params": n_params,
    }
    _CACHE["exec"] = state
    return state


def _run_cores(in_maps):
    ex = _get_exec()
    concat_in = [
        np.concatenate([np.asarray(m[name]) for m in in_maps], axis=0)
        for name in ex["in_names"]
    ]
    concat_zeros = [
        np.zeros((N_CORES * a.shape[0],) + tuple(a.shape[1:]), a.dtype)
        for a in ex["out_avals"]
    ]
    outs = ex["sharded"](*concat_in, *concat_zeros)
    name_to_i = {n: i for i, n in enumerate(ex["out_names"])}
    yi = name_to_i["y"]
    y_all = np.asarray(outs[yi]).reshape(N_CORES, L, D)
    return y_all


def kernel(x, Wqkv, Wo, s):
    in_maps = _make_in_maps(x, Wqkv, Wo, s)
    y_all = _run_cores(in_maps)
    out = y_all.sum(axis=0, dtype=np.float32)
    return out.reshape(1, L, D).astype(np.float32)
